# revision 3
# baseline (speedup 1.0000x reference)
"""DistMaps v4: host-side geometry, single fp16 packed input, K=5 pair
matmuls, pow-sqrt from PSUM on DVE/Pool, per-map single-engine mins,
per-tile tanh + DMA pipeline.

Device program (per core, SPMD over batch):
  G [6, 224*NP] fp16  (host-packed):
    per pair j: cols [224j, 224j+128) = lhsT rows {rsqA_t0, rsqA_t1,
    rsqB_t0, rsqB_t1, ones}; cols [224j+128, 224j+224) = rhs rows
    {indA_t0, indA_t1, indB_t0, indB_t1, csq}; row 5 = (t0, cs) int
    metadata for reg_load.
  matmul K=5 -> d^2 strips in PSUM -> pow 0.5 -> bf16 sqrt strips ->
  min into bf16 sqrt-space dmap (init 1000) -> per-tile tanh(2x) f32 -> DMA.
"""
from contextlib import ExitStack

import numpy as np

import concourse.bass as bass
import concourse.tile as tile
from concourse import bacc, mybir

F32 = mybir.dt.float32
F16 = mybir.dt.float16
BF16 = mybir.dt.bfloat16
I32 = mybir.dt.int32
AF = mybir.ActivationFunctionType
OP = mybir.AluOpType

B = 8
H = W = 512
NPTS = 24
NPM = 12
NT = 4
WS = 24            # strip column window (+-12 px; tanh(2*12/5) ~ 1-3e-5)
CM = WS // 2
SQBIG = 1000.0     # sqrt-space background
PEN = 3.0e4        # invalid-point penalty added to csq (fp16-safe)
PBLK = 224         # per-pair G block: 128 lhsT cols + 96 rhs cols
N_WARM = 4         # PE warmup matmuls


def build(npair0, npair1, qsize=2, pow_gran='quad', warm=N_WARM, split_first=True, tiles4=False, dm1_dma=False):
    NP = npair0 + npair1
    KR = 9 if tiles4 else 5          # lhsT rows
    RW = 192 if tiles4 else 96       # rhs cols per pair
    PB = 128 + RW                    # per-pair block in G
    NG = (2 if tiles4 else 4) * NP   # gint values
    nc = bacc.Bacc("TRN2", target_bir_lowering=False, debug=False,
                   num_devices=B)
    G = nc.dram_tensor("G", [KR + 1, PB * NP], F16, kind="ExternalInput").ap()
    y = nc.dram_tensor("y", [2, H, W], F32, kind="ExternalOutput").ap()

    with tile.TileContext(nc) as tc, ExitStack() as ctx:
        pool = ctx.enter_context(tc.tile_pool(name="sb", bufs=1))
        d_pool = ctx.enter_context(tc.tile_pool(name="dmap", bufs=1))
        psum = ctx.enter_context(tc.tile_pool(name="ps", bufs=6, space="PSUM"))
        psum_w = ctx.enter_context(tc.tile_pool(name="psw", bufs=1,
                                                space="PSUM"))
        sb_strips = ctx.enter_context(tc.tile_pool(name="sbs", bufs=6))
        out_pool = ctx.enter_context(tc.tile_pool(name="ob", bufs=8))

        # --- input DMAs (SP queue, issued immediately) ---
        Gs = pool.tile([KR, PB * NP], F16, tag="Gs")
        nc.sync.dma_start(Gs[:], G[0:KR, :])
        gif = pool.tile([1, NG], F16, tag="gif")
        nc.sync.dma_start(gif[:], G[KR:KR + 1, 0:NG])

        # --- constants / init (no input deps) ---
        wpow = pool.tile([1, 8], BF16, tag="wpow")
        nc.gpsimd.memset(wpow[:], 0.5)                       # Pool op 1
        # dummy pow on an immediately-ready tile: hoists the GPSIMD library
        # load into Pool's idle window
        nc.gpsimd.tensor_tensor(out=wpow[:], in0=wpow[:], in1=wpow[:],
                                op=OP.pow)
        wsrc = pool.tile([1, 516], F16, tag="wsrc")
        nc.gpsimd.memset(wsrc[:], 0.5)                       # Pool op 2
        halfs = pool.tile([128, (192 if tiles4 else 96) * qsize], BF16, tag="halfs")
        nc.gpsimd.memset(halfs[:], 0.5)                      # Pool op 3
        dm0 = d_pool.tile([128, NT * W], BF16, tag="d0")
        nc.vector.memset(dm0[:], SQBIG)                      # DVE op 1
        dm1 = d_pool.tile([128, NT * W], BF16, tag="d1")
        if dm1_dma:
            nc.sync.dma_start(dm1[:], dm0[:])
        else:
            nc.gpsimd.memset(dm1[:], SQBIG)                  # Pool op 3

        # --- PE warmup (keeps pstate streak alive until real matmuls) ---
        wps = psum_w.tile([128, 384], F32, tag="warm")
        for _ in range(warm):
            nc.tensor.matmul(wps[:], wsrc[0:1, 0:128], wsrc[0:1, 128:512],
                             start=True, stop=True, skip_group_check=True)

        # --- gint: fp16 metadata row -> int32, bulk register preload ---
        gi = pool.tile([1, NG], I32, tag="gi")
        nc.vector.tensor_copy(gi[:], gif[:])                 # DVE op 2
        reg_ctx = [ctx.enter_context(nc.vector.register(name=f"g{i}"))
                   for i in range(NG)]
        # TensorLoad supports at most 32 registers per instruction
        for lo in range(0, NG, 32):
            hi = min(lo + 32, NG)
            nc.vector.reg_load(reg_ctx[lo:hi], gi[0:1, lo:hi])

        d4s = [dm0[:].rearrange("p (t w) -> p t w", t=NT),
               dm1[:].rearrange("p (t w) -> p t w", t=NT)]

        # dynamic-sliced mins only work on DVE
        def eng(m):
            return nc.vector

        # quads: groups of <=2 pairs within one map
        quads = []
        for m, (lo, hi) in ((0, (0, npair0)), (1, (npair0, NP))):
            k = lo
            while k < hi:
                quads.append((m, list(range(k, min(k + qsize, hi)))))
                k += qsize

        for (m, prs) in quads:
            npr = len(prs)
            ncol = 96 * npr
            ps = psum.tile([128, RW * qsize], F32)
            for s, j in enumerate(prs):
                nc.tensor.matmul(ps[:, RW * s:RW * (s + 1)],
                                 Gs[0:KR, PB * j:PB * j + 128],
                                 Gs[0:KR, PB * j + 128:PB * (j + 1)],
                                 start=True, stop=True, skip_group_check=True)
            v = eng(m)
            ss = sb_strips.tile([128, RW * qsize], BF16)
            # PSUM -> SBUF on ACT (Copy shares the Tanh table), sqrt via
            # pow on GPSIMD (no PSUM access there), mins on DVE
            ncol = RW * npr
            nc.scalar.activation(ss[:, 0:ncol], ps[:, 0:ncol], AF.Copy)
            if pow_gran == 'quad':
                nc.gpsimd.tensor_tensor(out=ss[:, 0:ncol], in0=ss[:, 0:ncol],
                                        in1=halfs[:, 0:ncol], op=OP.pow)
            else:
                for s in range(npr):
                    o = RW * s
                    nc.gpsimd.tensor_tensor(out=ss[:, o:o + RW],
                                            in0=ss[:, o:o + RW],
                                            in1=halfs[:, 0:RW], op=OP.pow)
            for s, j in enumerate(prs):
                o = RW * s
                for k in range(2):
                    if tiles4:
                        rc = reg_ctx[2 * j + k]
                        csv = bass.make_scalar_value(rc, min_val=0,
                                                     max_val=W - WS)
                        dslice = d4s[m][:, 0:NT, bass.ds(csv, WS)]
                        off = o + 96 * k
                        v.tensor_tensor(
                            out=dslice,
                            in0=ss[:, off:off + 96].rearrange(
                                "p (c w) -> p c w", c=NT),
                            in1=dslice, op=OP.min)
                    else:
                        rt = reg_ctx[4 * j + 2 * k]
                        rc = reg_ctx[4 * j + 2 * k + 1]
                        t0v = bass.make_scalar_value(rt, min_val=0, max_val=2)
                        csv = bass.make_scalar_value(rc, min_val=0,
                                                     max_val=W - WS)
                        dslice = d4s[m][:, bass.ds(t0v, 2), bass.ds(csv, WS)]
                        off = o + 48 * k
                        v.tensor_tensor(
                            out=dslice,
                            in0=ss[:, off:off + 48].rearrange(
                                "p (c w) -> p c w", c=2),
                            in1=dslice, op=OP.min)
            if m == 0 and prs[-1] == npair0 - 1:
                # order fence: map1 mins (writers of dm1) must follow all
                # map0 mins; min(1000,1000) into dm1[0,0] is a numeric no-op
                nc.vector.tensor_tensor(out=dm1[0:1, 0:1], in0=dm0[0:1, 0:1],
                                        in1=dm1[0:1, 0:1], op=OP.min)
                # first tile split in half: starts the DMA stream earlier
                if split_first:
                    for (c0, c1) in ((0, 256), (256, 512)):
                        ob = out_pool.tile([128, 256], F32)
                        nc.scalar.activation(ob[:], dm0[:, c0:c1],
                                             AF.Tanh, scale=2.0)
                        nc.sync.dma_start(y[0, 0:128, c0:c1], ob[:])
                else:
                    ob = out_pool.tile([128, W], F32)
                    nc.scalar.activation(ob[:], dm0[:, 0:W], AF.Tanh, scale=2.0)
                    nc.sync.dma_start(y[0, 0:128, :], ob[:])
                for t in range(1, NT):
                    ob = out_pool.tile([128, W], F32)
                    nc.scalar.activation(ob[:], dm0[:, t * W:(t + 1) * W],
                                         AF.Tanh, scale=2.0)
                    nc.sync.dma_start(y[0, t * 128:(t + 1) * 128, :], ob[:])
        for t in range(NT):
            ob = out_pool.tile([128, W], F32)
            nc.scalar.activation(ob[:], dm1[:, t * W:(t + 1) * W],
                                 AF.Tanh, scale=2.0)
            nc.sync.dma_start(y[1, t * 128:(t + 1) * 128, :], ob[:])

    nc.compile()
    return nc


def _pairs_for(valid_idx):
    """Pair up point indices; odd count duplicates the last."""
    v = list(valid_idx)
    if not v:
        return []
    if len(v) % 2:
        v.append(v[-1])
    return [(v[i], v[i + 1]) for i in range(0, len(v), 2)]


def make_G(coords_b, pairs0, pairs1, tiles4=False):
    pairs = pairs0 + pairs1
    NP = len(pairs)
    if tiles4:
        KR, RW = 9, 192
    else:
        KR, RW = 5, 96
    PB = 128 + RW
    G = np.zeros((KR + 1, PB * NP), np.float32)
    for j, (ia, ib) in enumerate(pairs):
        base = PB * j
        G[KR - 1, base:base + 128] = 1.0  # ones row of lhsT
        for k, i in enumerate((ia, ib)):
            r = float(coords_b[i, 0])
            c = float(coords_b[i, 1])
            invalid = max(r, c) < 0
            cs = int(np.clip(np.floor(c) - CM, 0, W - WS))
            pen = PEN if invalid else 0.0
            csq = ((cs + np.arange(WS, dtype=np.float64) - c) * 0.2) ** 2 + pen
            if tiles4:
                rsq = ((np.arange(512, dtype=np.float64) - r) * 0.2) ** 2
                for t in range(4):
                    G[4 * k + t, base:base + 128] = rsq[128 * t:128 * (t + 1)]
                rb = base + 128 + 96 * k
                for t in range(4):
                    G[4 * k + t, rb + WS * t:rb + WS * (t + 1)] = 1.0
                    G[KR - 1, rb + WS * t:rb + WS * (t + 1)] = csq
                G[KR, 2 * j + k] = cs
            else:
                yrow = (r - 23.0) / 128.0
                t0 = int(yrow >= 1.0) + int(yrow >= 2.0)
                rows = np.arange(256, dtype=np.float64) + 128 * t0
                rsq = ((rows - r) * 0.2) ** 2
                G[2 * k + 0, base:base + 128] = rsq[:128]
                G[2 * k + 1, base:base + 128] = rsq[128:]
                rb = base + 128 + 48 * k
                G[2 * k + 0, rb:rb + WS] = 1.0
                G[2 * k + 1, rb + WS:rb + 2 * WS] = 1.0
                G[KR - 1, rb:rb + WS] = csq
                G[KR - 1, rb + WS:rb + 2 * WS] = csq
                G[KR, 4 * j + 2 * k + 0] = t0
                G[KR, 4 * j + 2 * k + 1] = cs
    return G.astype(np.float16)


_CACHE = {}


def _get_built(key):
    if key not in _CACHE:
        # (nc, consts) tuple: test harnesses index [0] for the Bacc module
        _CACHE[key] = (build(*key, **BUILD_OPTS), None)
    return _CACHE[key][0]

BUILD_OPTS = {'qsize': 1, 'split_first': False, 'warm': 3}


def kernel(x: np.ndarray, coords: np.ndarray) -> np.ndarray:
    from concourse.bass_utils import run_bass_kernel_spmd
    assert x.shape == (B, 3, H, W), x.shape
    assert coords.shape == (B, NPTS, 3), coords.shape
    coords = np.ascontiguousarray(coords, dtype=np.float32)

    val = coords[0, :, :2].max(axis=1) >= 0
    same = all(((coords[b, :, :2].max(axis=1) >= 0) == val).all()
               for b in range(B))
    if same:
        p0 = _pairs_for([i for i in range(NPM) if val[i]])
        p1 = _pairs_for([i for i in range(NPM, NPTS) if val[i]])
        if not p0:
            p0 = [(0, 0)]   # all-invalid map: penalty strips, no-op mins
        if not p1:
            p1 = [(NPM, NPM)]
    else:
        p0 = _pairs_for(list(range(NPM)))
        p1 = _pairs_for(list(range(NPM, NPTS)))

    nc = _get_built((len(p0), len(p1)))
    in_maps = [{"G": make_G(coords[b], p0, p1,
                            tiles4=BUILD_OPTS.get("tiles4", False))}
               for b in range(B)]
    last_err = None
    for _attempt in range(3):
        try:
            res = run_bass_kernel_spmd(nc, in_maps, list(range(B)))
            break
        except Exception as e:
            last_err = e
    else:
        raise last_err
    out = np.stack([res.results[b]["y"] for b in range(B)])
    return out.astype(np.float32)
